# revision 16
# baseline (speedup 1.0000x reference)
"""DenseQConv1D Trainium2 kernel.

Math: the reference computes, per output channel c and patch p (128-dim im2col
column of x, normalized):
    out[c,p] = sum_e sign(e) * (s_p^T (E @ R_c)[:128,:])_e^2 / ||p||^2
with R_c = kron of 9 RY(theta[c,q]) rotations and sign(e) = Z on the MSB qubit.
Because every RY factor is orthogonal and the measurement only touches qubit 0,
    out[c,p] = (cos t_c * p^T GZ p + sin t_c * p^T GX p) / ||p||^2
with GZ = F F^T - G G^T, GX = F G^T + G F^T for E128 = E[:128,:],
F = E128[:,:256], G = E128[:,256:].  For the CNOT-ring entangler E is a
permutation matrix, which makes GX == 0 and GZ == diag(s), s in {+-1}
(verified numerically: rel err 8.8e-7 vs the reference).  So
    out[c,l] = cos(theta[c,0]) * (sum_i s_i * p_i^2) / (sum_i p_i^2)

The per-channel weights (s_i * cos t_c, a [128,16] matrix) depend only on
theta / E — they are folded on the host (weight preprocessing); all of the
x-dependent compute (squares, the two partition reductions, reciprocal,
normalization/scaling) runs on device:
    pt[j*16+c, l] = x[c, l+j]           (im2col via windowed DMA APs)
    sq = pt*pt                          (DVE)
    n2[16,l] = ones^T  sq               (PE, K=128 M=16)
    zc[16,l] = Wzc^T   sq               (PE, Wzc[i,c] = s_i cos_c)
    out = zc * recip(n2)                (ACT reciprocal + DVE multiply)

Sharding: batch dimension across the 8 cores (core b computes x[b]).
"""

import numpy as np

B = 8
C_IN = 16
C_OUT = 16
L = 1024
K = 8
L_OUT = L - K + 1  # 1017
LP = 1024  # padded patch count per core (cols 1017:1024 are dummy)
P = 128  # patch vector length = C_IN*K = partitions

_CACHE = {}

# x DMA pieces (column ranges), alternating the two HWDGE queues so both
# stream concurrently; compute chunks are sized so the last chunk is small
# (short post-DMA tail: sq -> mm -> recip -> mul -> dma out).
DMA_PIECES = [(0, 256), (256, 256), (512, 256), (768, 249)]
CHUNKS = [(0, 512), (512, 256), (768, 256)]


def _build_nc():
    import bass_rust as _br
    import concourse.bacc as bacc
    import concourse.mybir as mybir
    import concourse.tile as tile

    f32 = mybir.dt.float32
    f32r = mybir.dt.float32r
    AF = mybir.ActivationFunctionType

    nc = bacc.Bacc("TRN2", target_bir_lowering=False, debug=False)

    def act_raw(out, in_, func, bias=0.0, scale=1.0):
        eng = nc.scalar
        ins = [
            eng.lower_ap(in_),
            mybir.ImmediateValue(dtype=mybir.dt.float32, value=bias),
            mybir.ImmediateValue(dtype=mybir.dt.float32, value=scale),
            mybir.ImmediateValue(dtype=mybir.dt.float32, value=0.0),
        ]
        return eng.add_instruction(
            mybir.InstActivation(
                name=nc.get_next_instruction_name(), func=func,
                ins=ins, outs=[eng.lower_ap(out)],
            )
        )

    x_ext = nc.declare_dram_parameter("x", [C_IN, L], f32, isOutput=False)
    w_ext = nc.declare_dram_parameter("w", [P, C_OUT], f32, isOutput=False)
    out_ext = nc.declare_dram_parameter("out", [C_OUT, LP], f32, isOutput=True)

    with tile.TileContext(nc) as tc, tc.tile_pool(name="sb", bufs=1) as sb, \
            tc.tile_pool(name="ps", bufs=1, space="PSUM") as ps:
        pt = sb.tile([P, LP], f32)
        sq = sb.tile([P, LP], f32r)
        wz = sb.tile([P, C_OUT], f32)
        wzr = sb.tile([P, C_OUT], f32r)
        w1 = sb.tile([P, C_OUT], f32)
        inv = sb.tile([C_OUT, LP], f32)
        outs = sb.tile([C_OUT, LP], f32)

        # x im2col windows: pieces alternate the two HWDGE queues so both
        # stream concurrently; tiny weight DMA on the (otherwise idle)
        # gpsimd SWDGE queue so it doesn't delay any x piece.
        nc.gpsimd.dma_start(wz[:], w_ext[:])
        engs = [nc.sync, nc.gpsimd, nc.sync, nc.scalar]
        for i, (lo, n) in enumerate(DMA_PIECES):
            xw = _br.AP(x_ext, lo, [[1, K], [L, C_IN], [1, n]])
            engs[i % len(engs)].dma_start(pt[:, lo : lo + n], xw)

        nc.vector.memset(pt[:, L_OUT:LP], 1.0)
        nc.vector.memset(w1[:], 1.0)

        for i, (lo, n) in enumerate(CHUNKS):
            s = slice(lo, lo + n)
            nc.vector.tensor_mul(sq[:, s], pt[:, s], pt[:, s])
            if i == 0:
                # f32r rounding of the weights (DVE cast, off critical path)
                nc.vector.tensor_copy(wzr[:], wz[:])
            pn = ps.tile([C_OUT, n], f32, name=f"pn{i}", tag=f"pn{i}")
            nc.tensor.matmul(
                pn[:], w1[:].bitcast(f32r), sq[:, s], start=True, stop=True
            )
            pz = ps.tile([C_OUT, n], f32, name=f"pz{i}", tag=f"pz{i}")
            nc.tensor.matmul(
                pz[:], wzr[:], sq[:, s], start=True, stop=True
            )
            act_raw(inv[:, s], pn[:], AF.Reciprocal, bias=1e-24)
            nc.vector.tensor_mul(outs[:, s], pz[:], inv[:, s])

        nc.sync.dma_start(out_ext[:, 0:512], outs[:, 0:512])
        nc.sync.dma_start(out_ext[:, 512:768], outs[:, 512:768])
        nc.sync.dma_start(out_ext[:, 768:LP], outs[:, 768:LP])

    nc.compile()
    return nc


def _host_weights(theta, ent):
    """Fold theta/entangle into the [128,16] stationary weight matrix."""
    e128 = ent[:P, :]
    f, g = e128[:, :256], e128[:, 256:]
    s_ref = (f * f).sum(1) - (g * g).sum(1)  # diag(GZ), ref patch order c*8+j
    s_dev = s_ref.reshape(C_IN, K).T.reshape(P)  # device order j*16+c
    cosv = np.cos(theta[:, 0])
    return np.ascontiguousarray(
        (s_dev[:, None] * cosv[None, :]).astype(np.float32)
    )


def kernel(**inputs):
    from concourse.bass_utils import run_bass_kernel_spmd

    x = np.ascontiguousarray(np.asarray(inputs["x"], dtype=np.float32))
    theta = np.ascontiguousarray(np.asarray(inputs["theta"], dtype=np.float32))
    ent = np.ascontiguousarray(
        np.asarray(inputs["entangle_matrix"], dtype=np.float32)
    )

    if "nc" not in _CACHE:
        _CACHE["nc"] = _build_nc()
    nc = _CACHE["nc"]

    w = _host_weights(theta, ent)
    in_maps = [
        {"x": np.ascontiguousarray(x[b]), "w": w} for b in range(B)
    ]
    res = run_bass_kernel_spmd(nc, in_maps, core_ids=list(range(B)))
    out = np.stack([res.results[b]["out"][:, :L_OUT] for b in range(B)], axis=0)
    return np.ascontiguousarray(out.astype(np.float32))
